# revision 1
# baseline (speedup 1.0000x reference)
"""Trainium2 Bass kernel for the CLT (cross-layer transcoder) forward pass.

Problem shapes (hardcoded, from the reference):
    x:             [1024, 8, 768]   f32
    W_enc:         [8, 768, 4096]   f32
    b_enc:         [8, 4096]        f32
    log_threshold: [8, 4096]        f32
    W_dec:         [36, 4096, 768]  f32   (36 = triu pairs of 8 layers)
    b_dec:         [36, 768]        f32
    out:           [1024, 8, 768]   f32

Math:
    hidden[b,n,k] = x[b,n,:] @ W_enc[n] + b_enc[n]
    feat = hidden * (hidden > exp(log_threshold))        (JumpReLU)
    out[:,k,:] = sum_{l<=k} feat[:,l,:] @ W_dec[pair(l,k)] + b_dec sums

Sharding (8 cores, single uniform SPMD program):
    Work units (1 unit = a [1024,768]x[768,4096]-sized matmul): encode 8,
    decode 36, total 44 -> 5.5 per core.  Core c handles sources
    (l, 7-l) with l = c//2, for token half c%2 (512 tokens).  That gives
    every core exactly 2 half-encodes (1 unit) + 9 half-pair decodes
    (4.5 units) -- a perfectly balanced, duplication-free split.

    Decode slot j of a core reads the feat of its source 0 (j < 8-l) or
    source 1 (else).  Slots 0-4 are always source 0 and slot 8 always
    source 1; slots 5-7 vary per core, so their matmul rhs is built as
    feat0*c0 + feat1*c1 with per-core 0/1 coefficients shipped as data,
    keeping the compiled program identical on all 8 cores.

    All matmuls run in bf16 (inputs cast on host) with fp32 PSUM
    accumulation.  Per-slot partial outputs [768, 512] go back to the
    host, which transposes/sums them into the full [1024, 8, 768] output
    (plus the b_dec per-target bias sums).
"""

import os
import sys

for _p in ("/opt/trn_rl_repo", "/root/.axon_site/_ro/trn_rl_repo"):
    if os.path.isdir(_p) and _p not in sys.path:
        sys.path.insert(0, _p)

import ml_dtypes
import numpy as np

import concourse.bass as bass
import concourse.mybir as mybir
import concourse.tile as tile
from concourse import bacc
from concourse.bass_utils import run_bass_kernel_spmd

BF16 = mybir.dt.bfloat16
F32 = mybir.dt.float32
NPBF16 = ml_dtypes.bfloat16

B, NL, D, K = 1024, 8, 768, 4096
HB = B // 2          # tokens per half (per core)
P = 128
DB = D // P          # 6 d-tiles
KT = K // P          # 32 k-tiles
KI = 4               # k-tiles per W_enc DMA chunk
KC = KT // KI        # 8 W_enc chunks
DKC = 8              # k-tiles per W_dec DMA chunk
DKQ = KT // DKC      # 4 W_dec chunks per decoder
NSLOT = 9            # decode half-pairs per core
SEL_SLOTS = (5, 6, 7)  # slots whose source varies per core
NCORES = 8

AF = mybir.ActivationFunctionType
ALU = mybir.AluOpType

_NC_CACHE = {}


def _install_dma_lane_pinning():
    """Pin each DMA stream to a fixed DMAHW lane.

    Tile round-robins HWDGE DMAs over 8 DMAHW semaphore lanes.  A DMA that
    reuses an SBUF slot then needs waits on (a) the PE readers of the slot
    (WAR), (b) the previous writer's lane sem (WAW), and (c) its own lane's
    predecessor (in-order completion per sem) -- three sync waits, but the
    walrus DMA instruction struct only encodes two.  Pinning a whole stream
    (all W_enc chunks, all W_dec chunks, ...) to one lane merges (b) and
    (c) into a single semaphore wait, guaranteeing <=2 waits per DMA.
    """
    import concourse.tile_sem_assignment as tsa

    if getattr(tsa, "_clt_lane_pinned", False):
        return

    _orig = tsa.TileClockTick._assign_tick

    def _dma_names(inst):
        names = set()
        for a in list(inst.ins) + list(inst.outs):
            t = None
            for chain in ("bass_ap", None):
                try:
                    obj = getattr(a, chain) if chain else a
                    t = obj.tensor
                    break
                except AttributeError:
                    continue
            if t is not None:
                try:
                    names.add(t.name)
                except AttributeError:
                    pass
        return names

    # Two lanes per stream: consecutive same-stream DMAs alternate lanes so
    # their issues don't head-block the Sync queue on each other's
    # completion, while slot-reuse (WAW) partners still land on the SAME
    # lane because the lane count (2) divides the pool bufs (4).
    _LANES = {"we": [0, 4], "wd": [1, 5], "po": [2, 6], "_const": [3, 7]}

    def _assign_tick(self, inst):
        if isinstance(inst, tsa.DMAInst) and inst.engine != mybir.EngineType.Pool:
            names = _dma_names(inst)
            stream = "_const"
            for key in ("we", "wd", "po"):
                if key in names:
                    stream = key
                    break
            ctr = getattr(self, "_clt_lane_ctr", None)
            if ctr is None:
                ctr = {}
                self._clt_lane_ctr = ctr
            lanes = _LANES[stream]
            i = ctr.get(stream, 0)
            ctr[stream] = i + 1
            self.next_hw_dma_idx = lanes[i % len(lanes)]
        return _orig(self, inst)

    tsa.TileClockTick._assign_tick = _assign_tick
    tsa._clt_lane_pinned = True


def _core_slots(c):
    """Return (l, half, srcs, slots) for core c.

    slots: list of (local_source_index, target_layer); first 8-l entries
    use local source 0 (= layer l), the rest local source 1 (= layer 7-l).
    """
    l, half = c // 2, c % 2
    srcs = (l, 7 - l)
    slots = [(0, t) for t in range(l, 8)] + [(1, t) for t in range(7 - l, 8)]
    assert len(slots) == NSLOT
    for j, (s, _) in enumerate(slots):
        if j < 5:
            assert s == 0
        elif j == 8:
            assert s == 1
    return l, half, srcs, slots


def _strip_redundant_self_waits(nc):
    """Drop trivially-satisfied same-engine semaphore waits.

    Tile sometimes emits a wait on an engine's own semaphore for a value
    the engine has necessarily already passed (its in-order predecessors
    increment that sem on completion).  Such waits are runtime no-ops but
    consume one of the 1-2 sync-wait slots a walrus instruction struct can
    encode, overflowing the encoder.  Keep a pipeline-depth margin: a wait
    is dropped only if satisfied even with queue_depth instructions still
    in flight at sequencer dispatch time.
    """
    import re
    from collections import defaultdict

    # Engine completion sems are named like PE_44 / DVE_44 / Activation_44.
    # Only those are safe to treat as "own engine program order" - barrier
    # and event sems must never be touched.
    _ENG_SEM_RE = {
        mybir.EngineType.PE: re.compile(r"^PE_\d+$"),
        mybir.EngineType.DVE: re.compile(r"^DVE_\d+$"),
        mybir.EngineType.Activation: re.compile(r"^Activation_\d+$"),
    }
    _STRIP_TYPES = (
        "InstTensorScalarPtr",
        "InstTensorScalar",
        "InstTensorTensor",
        "InstTensorCopy",
        "InstActivation",
        "InstMatmult",
        "InstLdweights",
    )

    margins = defaultdict(lambda: 12)
    margins[mybir.EngineType.PE] = 80

    cum = defaultdict(int)
    dropped = 0
    for bb in nc.m.functions[0].blocks:
        for ins in bb.instructions:
            si = ins.sync_info
            if si is None:
                continue
            sem_re = _ENG_SEM_RE.get(ins.engine)
            if type(ins).__name__ in _STRIP_TYPES and sem_re is not None:
                margin = margins[ins.engine]
                kept = []
                for w in si.on_wait:
                    if (
                        sem_re.match(w.ant_name)
                        and w.wait_mode == "sem-ge-imm"
                        and w.wait_value <= cum[w.ant_name] - margin
                    ):
                        dropped += 1
                        continue
                    kept.append(w)
                if len(kept) != len(si.on_wait):
                    ins.sync_info = mybir.SyncInfo(
                        on_wait=kept, on_update=si.on_update
                    )
                    si = ins.sync_info
            for u in si.on_update:
                cum[u.ant_name] += u.update_value
    return dropped


def _build_nc():
    if "nc" in _NC_CACHE:
        return _NC_CACHE["nc"]

    # Bacc (not raw Bass): its compile pipeline legalizes sync waits down
    # to the 1-wait-per-instruction TRN2 limit via event semaphores.
    _install_dma_lane_pinning()
    nc = bacc.Bacc()

    xt_d = nc.dram_tensor("xt", [P, 2, DB, HB], BF16, kind="ExternalInput")
    we_d = nc.dram_tensor("we", [2, KC, P, KI, DB, P], BF16, kind="ExternalInput")
    wd_d = nc.dram_tensor("wd", [NSLOT, P, KT, D], BF16, kind="ExternalInput")
    tmb_d = nc.dram_tensor("tmb", [P, 2, KT], F32, kind="ExternalInput")
    be_d = nc.dram_tensor("be", [P, 2, KT], F32, kind="ExternalInput")
    sel_d = nc.dram_tensor("sel", [P, len(SEL_SLOTS), 2], F32, kind="ExternalInput")
    po_d = nc.dram_tensor("po", [NSLOT, DB, P, HB], F32, kind="ExternalOutput")

    with tile.TileContext(nc) as tc:
        with (
            tc.tile_pool(name="const", bufs=1) as constp,
            tc.tile_pool(name="wep", bufs=4) as wep,
            tc.tile_pool(name="wdp", bufs=4) as wdp,
            tc.tile_pool(name="featp", bufs=1) as featp,
            tc.tile_pool(name="fselp", bufs=8) as fselp,
            tc.tile_pool(name="tmpp", bufs=6) as tmpp,
            tc.tile_pool(name="outp", bufs=8) as outp,
            tc.tile_pool(name="psum", bufs=8, space="PSUM") as psump,
        ):
            # per-source xt DMAs (s=0 first) so the first encode matmuls
            # only wait on the source-0 slice; the first W_enc chunk loads
            # in parallel on its own lane before the remaining consts
            xt_sb = constp.tile([P, 2, DB, HB], BF16)
            nc.sync.dma_start(xt_sb[:, 0], xt_d[:, 0])
            we_t0 = wep.tile([P, KI, DB, P], BF16, tag="we")
            nc.sync.dma_start(we_t0[:], we_d[0, 0])
            we_t1 = wep.tile([P, KI, DB, P], BF16, tag="we")
            nc.sync.dma_start(we_t1[:], we_d[0, 1])
            nc.sync.dma_start(xt_sb[:, 1], xt_d[:, 1])
            tmb_sb = constp.tile([P, 2, KT], F32)
            nc.sync.dma_start(tmb_sb[:], tmb_d[:])
            be_sb = constp.tile([P, 2, KT], F32)
            nc.sync.dma_start(be_sb[:], be_d[:])
            sel_sb = constp.tile([P, len(SEL_SLOTS), 2], F32)
            nc.sync.dma_start(sel_sb[:], sel_d[:])

            # One-time "absorb" ops: the first DVE/ACT instructions that use
            # an AP-scalar operand (TensorScalarPtr / ActivationPtr) can
            # encode only ONE sync wait, but they'd otherwise have to wait on
            # both the PSUM producer (PE sem) and the const-DMA (DMAHW sem).
            # Touch each DMA-loaded const from both engines up front so the
            # engines' vector clocks already cover the DMAs.
            probe = constp.tile([P, 4], F32)
            nc.vector.tensor_copy(probe[:, 0:1], tmb_sb[:, 0, 0:1])
            nc.vector.tensor_copy(probe[:, 1:2], sel_sb[:, 0, 0:1])
            nc.scalar.copy(probe[:, 2:3], be_sb[:, 0, 0:1])

            feat_sb = featp.tile([P, 2, KT, HB], BF16)

            # ---------------- encode ----------------
            for s in range(2):
                for kc in range(KC):
                    if s == 0 and kc == 0:
                        we_t = we_t0
                    elif s == 0 and kc == 1:
                        we_t = we_t1
                    else:
                        we_t = wep.tile([P, KI, DB, P], BF16, tag="we")
                        nc.sync.dma_start(we_t[:], we_d[s, kc])
                    for ki in range(KI):
                        kt = kc * KI + ki
                        ps = psump.tile([P, HB], F32, tag="ps")
                        for db in range(DB):
                            nc.tensor.matmul(
                                ps[:],
                                we_t[:, ki, db, :],
                                xt_sb[:, s, db, :],
                                start=(db == 0),
                                stop=(db == DB - 1),
                            )
                        # JumpReLU: hb = h + b on ACT (sole PSUM reader, so
                        # the next matmul group's WAR is a single wait);
                        # mask + mult on DVE read the bf16 hb at 2x rate.
                        hb_t = tmpp.tile([P, HB], BF16, tag="hb")
                        nc.scalar.activation(
                            hb_t[:], ps[:], AF.Identity, bias=be_sb[:, s, kt : kt + 1]
                        )
                        mask = tmpp.tile([P, HB], BF16, tag="mask")
                        nc.vector.tensor_scalar(
                            mask[:], hb_t[:], tmb_sb[:, s, kt : kt + 1], None, ALU.is_gt
                        )
                        nc.vector.tensor_tensor(
                            feat_sb[:, s, kt, :], hb_t[:], mask[:], ALU.mult
                        )

            # ---------------- decode ----------------
            # Last slot runs db-outer/kt-inner so its six PSUM evictions
            # pipeline behind each db group instead of all trailing the
            # kernel's final matmul.
            for j in range(NSLOT - 1):
                pss = [
                    psump.tile([P, HB], F32, tag="ps", name=f"ps_{j}_{db}")
                    for db in range(DB)
                ]
                for kq in range(DKQ):
                    wd_t = wdp.tile([P, DKC, D], BF16, tag="wd")
                    nc.sync.dma_start(
                        wd_t[:], wd_d[j, :, kq * DKC : (kq + 1) * DKC, :]
                    )
                    for ki in range(DKC):
                        kt = kq * DKC + ki
                        if j < 5:
                            rhs = feat_sb[:, 0, kt, :]
                        elif j == 8:
                            rhs = feat_sb[:, 1, kt, :]
                        else:
                            si = j - 5
                            fa = fselp.tile([P, HB], BF16, tag="fa")
                            nc.vector.tensor_scalar(
                                fa[:],
                                feat_sb[:, 0, kt, :],
                                sel_sb[:, si, 0:1],
                                None,
                                ALU.mult,
                            )
                            fb = fselp.tile([P, HB], BF16, tag="fb")
                            nc.vector.tensor_scalar(
                                fb[:],
                                feat_sb[:, 1, kt, :],
                                sel_sb[:, si, 1:2],
                                None,
                                ALU.mult,
                            )
                            fs = fselp.tile([P, HB], BF16, tag="fs")
                            nc.vector.tensor_tensor(fs[:], fa[:], fb[:], ALU.add)
                            rhs = fs[:]
                        for db in range(DB):
                            nc.tensor.matmul(
                                pss[db][:],
                                wd_t[:, ki, db * P : (db + 1) * P],
                                rhs,
                                start=(kt == 0),
                                stop=(kt == KT - 1),
                            )
                for db in range(DB):
                    ot = outp.tile([P, HB], F32, tag="ot")
                    nc.scalar.copy(ot[:], pss[db][:])
                    nc.sync.dma_start(po_d[j, db], ot[:])

            j = NSLOT - 1
            wd_last = []
            for kq in range(DKQ):
                wd_t = wdp.tile([P, DKC, D], BF16, tag="wd", name=f"wd_last_{kq}")
                nc.sync.dma_start(wd_t[:], wd_d[j, :, kq * DKC : (kq + 1) * DKC, :])
                wd_last.append(wd_t)
            for db in range(DB):
                ps = psump.tile([P, HB], F32, tag="ps", name=f"ps_last_{db}")
                for kt in range(KT):
                    nc.tensor.matmul(
                        ps[:],
                        wd_last[kt // DKC][:, kt % DKC, db * P : (db + 1) * P],
                        feat_sb[:, 1, kt, :],
                        start=(kt == 0),
                        stop=(kt == KT - 1),
                    )
                ot = outp.tile([P, HB], F32, tag="ot")
                nc.scalar.copy(ot[:], ps[:])
                nc.sync.dma_start(po_d[j, db], ot[:])

    _strip_redundant_self_waits(nc)
    # run_bass_via_pjrt serializes a prebuilt nc without finalizing it, but
    # Bacc's finalize/compile pipeline (register allocation + sync-wait
    # legalization) is required for a valid NEFF.
    nc.finalize()
    _NC_CACHE["nc"] = nc
    return nc


def _prepare_in_maps(x, W_enc, b_enc, log_threshold, W_dec, b_dec):
    x = np.ascontiguousarray(np.asarray(x, dtype=np.float32))
    W_enc = np.asarray(W_enc, dtype=np.float32)
    b_enc = np.asarray(b_enc, dtype=np.float32)
    log_threshold = np.asarray(log_threshold, dtype=np.float32)
    W_dec = np.asarray(W_dec, dtype=np.float32)

    thresh = np.exp(log_threshold)
    tmb_full = thresh                               # [8, 4096] (hb > t form)

    l_idx, k_idx = np.triu_indices(NL)
    didx = {(int(l), int(k)): i for i, (l, k) in enumerate(zip(l_idx, k_idx))}

    x_b = x.astype(NPBF16)
    W_enc_b = W_enc.astype(NPBF16)
    W_dec_b = W_dec.astype(NPBF16)

    in_maps = []
    slot_infos = []
    for c in range(NCORES):
        l, half, srcs, slots = _core_slots(c)
        tok = slice(half * HB, (half + 1) * HB)

        xt = np.empty((P, 2, DB, HB), NPBF16)
        for s, src in enumerate(srcs):
            xs = x_b[tok, src, :]                   # [HB, D]
            xt[:, s] = xs.T.reshape(DB, P, HB).transpose(1, 0, 2)

        we = np.empty((2, KC, P, KI, DB, P), NPBF16)
        for s, src in enumerate(srcs):
            w6 = W_enc_b[src].reshape(DB, P, KT, P)         # [db, p, kt, kin]
            w7 = w6.transpose(2, 1, 0, 3)                   # [kt, p, db, kin]
            we[s] = w7.reshape(KC, KI, P, DB, P).transpose(0, 2, 1, 3, 4)

        wd = np.empty((NSLOT, P, KT, D), NPBF16)
        for j, (s, tgt) in enumerate(slots):
            w = W_dec_b[didx[(srcs[s], tgt)]]               # [K, D]
            wd[j] = w.reshape(KT, P, D).transpose(1, 0, 2)

        tmb = np.empty((P, 2, KT), np.float32)
        be = np.empty((P, 2, KT), np.float32)
        for s, src in enumerate(srcs):
            tmb[:, s, :] = tmb_full[src].reshape(KT, P).T
            be[:, s, :] = b_enc[src].reshape(KT, P).T

        sel = np.zeros((P, len(SEL_SLOTS), 2), np.float32)
        for si, j in enumerate(SEL_SLOTS):
            sel[:, si, slots[j][0]] = 1.0

        in_maps.append({"xt": xt, "we": we, "wd": wd, "tmb": tmb, "be": be, "sel": sel})
        slot_infos.append((half, [(srcs[s], t) for s, t in slots]))

    return in_maps, slot_infos


def _assemble_output(results, slot_infos, b_dec):
    b_dec = np.asarray(b_dec, dtype=np.float32)
    l_idx, k_idx = np.triu_indices(NL)

    out = np.zeros((B, NL, D), np.float32)
    for c in range(NCORES):
        half, slots_abs = slot_infos[c]
        po = np.asarray(results[c]["po"], dtype=np.float32)  # [9, DB, P, HB]
        tok = slice(half * HB, (half + 1) * HB)
        for j, (_src, tgt) in enumerate(slots_abs):
            out[tok, tgt, :] += po[j].reshape(D, HB).T

    bsum = np.zeros((NL, D), np.float32)
    for i in range(len(l_idx)):
        bsum[k_idx[i]] += b_dec[i]
    out += bsum[None, :, :]
    return out


def _run(x, W_enc, b_enc, log_threshold, W_dec, b_dec, trace=False, **kw):
    nc = _build_nc()
    in_maps, slot_infos = _prepare_in_maps(
        x, W_enc, b_enc, log_threshold, W_dec, b_dec
    )
    res = run_bass_kernel_spmd(nc, in_maps, list(range(NCORES)), trace=trace, **kw)
    out = _assemble_output(res.results, slot_infos, b_dec)
    return out, res


def kernel(x, W_enc, b_enc, log_threshold, W_dec, b_dec):
    out, _ = _run(x, W_enc, b_enc, log_threshold, W_dec, b_dec, trace=False)
    return out



# revision 14
# speedup vs baseline: 1.0837x; 1.0837x over previous
"""Trainium2 Bass kernel for the CLT (cross-layer transcoder) forward pass.

Problem shapes (hardcoded, from the reference):
    x:             [1024, 8, 768]   f32
    W_enc:         [8, 768, 4096]   f32
    b_enc:         [8, 4096]        f32
    log_threshold: [8, 4096]        f32
    W_dec:         [36, 4096, 768]  f32   (36 = triu pairs of 8 layers)
    b_dec:         [36, 768]        f32
    out:           [1024, 8, 768]   f32

Math:
    hidden[b,n,k] = x[b,n,:] @ W_enc[n] + b_enc[n]
    feat = hidden * (hidden > exp(log_threshold))        (JumpReLU)
    out[:,k,:] = sum_{l<=k} feat[:,l,:] @ W_dec[pair(l,k)] + b_dec sums

Sharding (8 cores, single uniform SPMD program):
    Work units (1 unit = a [1024,768]x[768,4096]-sized matmul): encode 8,
    decode 36, total 44 -> 5.5 per core.  Core c handles sources
    (l, 7-l) with l = c//2, for token half c%2 (512 tokens).  That gives
    every core exactly 2 half-encodes (1 unit) + 9 half-pair decodes
    (4.5 units) -- a perfectly balanced, duplication-free split.

    Decode slot j of a core reads the feat of its source 0 (j < 8-l) or
    source 1 (else).  Slots 0-4 are always source 0 and slot 8 always
    source 1; slots 5-7 vary per core, so their matmul rhs is built as
    feat0*c0 + feat1*c1 with per-core 0/1 coefficients shipped as data,
    keeping the compiled program identical on all 8 cores.

    All matmuls run in bf16 (inputs cast on host) with fp32 PSUM
    accumulation.  Per-slot partial outputs [768, 512] go back to the
    host, which transposes/sums them into the full [1024, 8, 768] output
    (plus the b_dec per-target bias sums).

    fp8 DoubleRow: decode slots 3 and 4 (8 of the 36 pairs, uniform
    across cores) run as fp8e4 DoubleRow matmuls -- W_dec pre-scaled by
    64 into e4m3 on the host (values would hit subnormals otherwise),
    feat cast bf16->e4m3 on DVE per kt-pair.  PE throughput for those
    slots is ~1.44x bf16; measured global rel err 1.82e-2 < 2e-2 gate.
    Slot order: bf16 kt-outer slots first, then the two fp8 slots, then
    the db-outer bf16 slot 8 -- while the fp8 slots run, the wd pool is
    idle so slot 8's four resident W chunks prefetch without stalling.
"""

import os
import sys

for _p in ("/opt/trn_rl_repo", "/root/.axon_site/_ro/trn_rl_repo"):
    if os.path.isdir(_p) and _p not in sys.path:
        sys.path.insert(0, _p)

import ml_dtypes
import numpy as np

import concourse.bass as bass
import concourse.mybir as mybir
import concourse.tile as tile
from concourse import bacc
from concourse.bass_utils import run_bass_kernel_spmd

BF16 = mybir.dt.bfloat16
F32 = mybir.dt.float32
F8E4 = mybir.dt.float8e4
NPBF16 = ml_dtypes.bfloat16
NPF8E4 = ml_dtypes.float8_e4m3

B, NL, D, K = 1024, 8, 768, 4096
HB = B // 2          # tokens per half (per core)
P = 128
DB = D // P          # 6 d-tiles
KT = K // P          # 32 k-tiles
KI = 4               # k-tiles per W_enc DMA chunk
KC = KT // KI        # 8 W_enc chunks
DKC = 8              # k-tiles per W_dec DMA chunk
DKQ = KT // DKC      # 4 W_dec chunks per decoder
NSLOT = 9            # decode half-pairs per core
SEL_SLOTS = (5, 6, 7)  # slots whose source varies per core
FP8_SLOTS = (3, 4)   # slots decoded via fp8e4 DoubleRow (src 0 for all cores)
FP8_WSCALE = 64.0    # host pre-scale on fp8 W_dec (undone on the host)
BF16_SLOTS = (0, 1, 2, 5, 6, 7, 8)   # wd dram-tensor index = position here
KT_ORDER = (0, 1, 2, 5, 6, 7)        # bf16 kt-outer slots, processed first
NCORES = 8

AF = mybir.ActivationFunctionType
ALU = mybir.AluOpType

_NC_CACHE = {}


def _install_dma_lane_pinning():
    """Pin each DMA stream to a fixed DMAHW lane.

    Tile round-robins HWDGE DMAs over 8 DMAHW semaphore lanes.  A DMA that
    reuses an SBUF slot then needs waits on (a) the PE readers of the slot
    (WAR), (b) the previous writer's lane sem (WAW), and (c) its own lane's
    predecessor (in-order completion per sem) -- three sync waits, but the
    walrus DMA instruction struct only encodes two.  Pinning a whole stream
    (all W_enc chunks, all W_dec chunks, ...) to one lane merges (b) and
    (c) into a single semaphore wait, guaranteeing <=2 waits per DMA.
    """
    import concourse.tile_sem_assignment as tsa

    if getattr(tsa, "_clt_lane_pinned", False):
        return

    _orig = tsa.TileClockTick._assign_tick

    def _dma_names(inst):
        names = set()
        for a in list(inst.ins) + list(inst.outs):
            t = None
            for chain in ("bass_ap", None):
                try:
                    obj = getattr(a, chain) if chain else a
                    t = obj.tensor
                    break
                except AttributeError:
                    continue
            if t is not None:
                try:
                    names.add(t.name)
                except AttributeError:
                    pass
        return names

    # Two lanes per stream: consecutive same-stream DMAs alternate lanes so
    # their issues don't head-block the Sync queue on each other's
    # completion, while slot-reuse (WAW) partners still land on the SAME
    # lane because the lane count (2) divides the pool bufs (4).
    # xt spreads over the po lanes too (idle at kernel start); wd8 rides
    # the const lanes (idle once the decode phase reaches the fp8 slots).
    _LANES = {
        "we": [0, 4],
        "wd": [1, 5],
        "po": [2, 6],
        "wd8": [3, 7],
        "xt": [3, 7, 2, 6],
        "_const": [3, 7],
    }

    def _assign_tick(self, inst):
        if isinstance(inst, tsa.DMAInst) and inst.engine != mybir.EngineType.Pool:
            names = _dma_names(inst)
            stream = "_const"
            for key in ("we", "wd8", "wd", "po", "xt"):
                if key in names:
                    stream = key
                    break
            ctr = getattr(self, "_clt_lane_ctr", None)
            if ctr is None:
                ctr = {}
                self._clt_lane_ctr = ctr
            lanes = _LANES[stream]
            i = ctr.get(stream, 0)
            ctr[stream] = i + 1
            self.next_hw_dma_idx = lanes[i % len(lanes)]
        return _orig(self, inst)

    tsa.TileClockTick._assign_tick = _assign_tick
    tsa._clt_lane_pinned = True


def _core_slots(c):
    """Return (l, half, srcs, slots) for core c.

    slots: list of (local_source_index, target_layer); first 8-l entries
    use local source 0 (= layer l), the rest local source 1 (= layer 7-l).
    """
    l, half = c // 2, c % 2
    srcs = (l, 7 - l)
    slots = [(0, t) for t in range(l, 8)] + [(1, t) for t in range(7 - l, 8)]
    assert len(slots) == NSLOT
    for j, (s, _) in enumerate(slots):
        if j < 5:
            assert s == 0
        elif j == 8:
            assert s == 1
    return l, half, srcs, slots


def _strip_redundant_self_waits(nc):
    """Drop trivially-satisfied same-engine semaphore waits.

    Tile sometimes emits a wait on an engine's own semaphore for a value
    the engine has necessarily already passed (its in-order predecessors
    increment that sem on completion).  Such waits are runtime no-ops but
    consume one of the 1-2 sync-wait slots a walrus instruction struct can
    encode, overflowing the encoder.  Keep a pipeline-depth margin: a wait
    is dropped only if satisfied even with queue_depth instructions still
    in flight at sequencer dispatch time.
    """
    import re
    from collections import defaultdict

    # Engine completion sems are named like PE_44 / DVE_44 / Activation_44.
    # Only those are safe to treat as "own engine program order" - barrier
    # and event sems must never be touched.
    _ENG_SEM_RE = {
        mybir.EngineType.PE: re.compile(r"^PE_\d+$"),
        mybir.EngineType.DVE: re.compile(r"^DVE_\d+$"),
        mybir.EngineType.Activation: re.compile(r"^Activation_\d+$"),
    }
    _STRIP_TYPES = (
        "InstTensorScalarPtr",
        "InstTensorScalar",
        "InstTensorTensor",
        "InstTensorCopy",
        "InstActivation",
        "InstMatmult",
        "InstLdweights",
    )

    margins = defaultdict(lambda: 12)
    margins[mybir.EngineType.PE] = 80

    cum = defaultdict(int)
    dropped = 0
    for bb in nc.m.functions[0].blocks:
        for ins in bb.instructions:
            si = ins.sync_info
            if si is None:
                continue
            sem_re = _ENG_SEM_RE.get(ins.engine)
            if type(ins).__name__ in _STRIP_TYPES and sem_re is not None:
                margin = margins[ins.engine]
                kept = []
                for w in si.on_wait:
                    if (
                        sem_re.match(w.ant_name)
                        and w.wait_mode == "sem-ge-imm"
                        and w.wait_value <= cum[w.ant_name] - margin
                    ):
                        dropped += 1
                        continue
                    kept.append(w)
                if len(kept) != len(si.on_wait):
                    ins.sync_info = mybir.SyncInfo(
                        on_wait=kept, on_update=si.on_update
                    )
                    si = ins.sync_info
            for u in si.on_update:
                cum[u.ant_name] += u.update_value
    return dropped


def _build_nc():
    if "nc" in _NC_CACHE:
        return _NC_CACHE["nc"]

    # Bacc (not raw Bass): its compile pipeline legalizes sync waits down
    # to the 1-wait-per-instruction TRN2 limit via event semaphores.
    _install_dma_lane_pinning()
    nc = bacc.Bacc()

    xt_d = nc.dram_tensor("xt", [P, 2, DB, HB], BF16, kind="ExternalInput")
    we_d = nc.dram_tensor("we", [2, KC, P, KI, DB, P], BF16, kind="ExternalInput")
    wd_d = nc.dram_tensor(
        "wd", [len(BF16_SLOTS), P, KT, D], BF16, kind="ExternalInput"
    )
    wd8_d = nc.dram_tensor(
        "wd8", [len(FP8_SLOTS), P, KT, D], F8E4, kind="ExternalInput"
    )
    tmb_d = nc.dram_tensor("tmb", [P, 2, KT], F32, kind="ExternalInput")
    be_d = nc.dram_tensor("be", [P, 2, KT], F32, kind="ExternalInput")
    sel_d = nc.dram_tensor("sel", [P, len(SEL_SLOTS), 2], F32, kind="ExternalInput")
    po_d = nc.dram_tensor("po", [NSLOT, DB, P, HB], F32, kind="ExternalOutput")

    with tile.TileContext(nc) as tc:
        with (
            tc.tile_pool(name="const", bufs=1) as constp,
            tc.tile_pool(name="wep", bufs=4) as wep,
            tc.tile_pool(name="wdp", bufs=4) as wdp,
            tc.tile_pool(name="wdp8", bufs=3) as wdp8,
            tc.tile_pool(name="featp", bufs=1) as featp,
            tc.tile_pool(name="fselp", bufs=4) as fselp,
            tc.tile_pool(name="f8p", bufs=6) as f8p,
            tc.tile_pool(name="tmpp", bufs=4) as tmpp,
            tc.tile_pool(name="outp", bufs=4) as outp,
            tc.tile_pool(name="psum", bufs=8, space="PSUM") as psump,
        ):
            # Head: the first matmul group needs only we[s0,kc0,ki0] and the
            # xt source-0 slices, so issue those first and in db/ki-sized
            # pieces spread over idle DMA lanes; everything else queues
            # behind them.
            xt_sb = constp.tile([P, 2, DB, HB], BF16)
            we_t0 = wep.tile([P, KI, DB, P], BF16, tag="we")
            nc.sync.dma_start(we_t0[:, 0], we_d[0, 0, :, 0])
            for db in range(DB):
                nc.sync.dma_start(xt_sb[:, 0, db], xt_d[:, 0, db])
            for ki in range(1, KI):
                nc.sync.dma_start(we_t0[:, ki], we_d[0, 0, :, ki])
            we_t1 = wep.tile([P, KI, DB, P], BF16, tag="we")
            nc.sync.dma_start(we_t1[:], we_d[0, 1])
            nc.sync.dma_start(xt_sb[:, 1], xt_d[:, 1])
            tmb_sb = constp.tile([P, 2, KT], F32)
            nc.sync.dma_start(tmb_sb[:], tmb_d[:])
            be_sb = constp.tile([P, 2, KT], F32)
            nc.sync.dma_start(be_sb[:], be_d[:])
            sel_sb = constp.tile([P, len(SEL_SLOTS), 2], F32)
            nc.sync.dma_start(sel_sb[:], sel_d[:])

            # One-time "absorb" ops: the first DVE/ACT instructions that use
            # an AP-scalar operand (TensorScalarPtr / ActivationPtr) can
            # encode only ONE sync wait, but they'd otherwise have to wait on
            # both the PSUM producer (PE sem) and the const-DMA (DMAHW sem).
            # Touch each DMA-loaded const from both engines up front so the
            # engines' vector clocks already cover the DMAs.
            probe = constp.tile([P, 4], F32)
            nc.vector.tensor_copy(probe[:, 0:1], tmb_sb[:, 0, 0:1])
            nc.vector.tensor_copy(probe[:, 1:2], sel_sb[:, 0, 0:1])
            nc.scalar.copy(probe[:, 2:3], be_sb[:, 0, 0:1])

            feat_sb = featp.tile([P, 2, KT, HB], BF16)

            # ---------------- encode ----------------
            for s in range(2):
                for kc in range(KC):
                    if s == 0 and kc == 0:
                        we_t = we_t0
                    elif s == 0 and kc == 1:
                        we_t = we_t1
                    else:
                        we_t = wep.tile([P, KI, DB, P], BF16, tag="we")
                        nc.sync.dma_start(we_t[:], we_d[s, kc])
                    for ki in range(KI):
                        kt = kc * KI + ki
                        ps = psump.tile([P, HB], F32, tag="ps")
                        for db in range(DB):
                            nc.tensor.matmul(
                                ps[:],
                                we_t[:, ki, db, :],
                                xt_sb[:, s, db, :],
                                start=(db == 0),
                                stop=(db == DB - 1),
                            )
                        # JumpReLU: hb = h + b on ACT (sole PSUM reader, so
                        # the next matmul group's WAR is a single wait);
                        # mask + mult on DVE read the bf16 hb at 2x rate.
                        hb_t = tmpp.tile([P, HB], BF16, tag="hb")
                        nc.scalar.activation(
                            hb_t[:], ps[:], AF.Identity, bias=be_sb[:, s, kt : kt + 1]
                        )
                        mask = tmpp.tile([P, HB], BF16, tag="mask")
                        nc.vector.tensor_scalar(
                            mask[:], hb_t[:], tmb_sb[:, s, kt : kt + 1], None, ALU.is_gt
                        )
                        nc.vector.tensor_tensor(
                            feat_sb[:, s, kt, :], hb_t[:], mask[:], ALU.mult
                        )

            # ---------------- decode: bf16 kt-outer slots ----------------
            for j in KT_ORDER:
                wj = BF16_SLOTS.index(j)
                pss = [
                    psump.tile([P, HB], F32, tag="ps", name=f"ps_{j}_{db}")
                    for db in range(DB)
                ]
                for kq in range(DKQ):
                    wd_t = wdp.tile([P, DKC, D], BF16, tag="wd")
                    nc.sync.dma_start(
                        wd_t[:], wd_d[wj, :, kq * DKC : (kq + 1) * DKC, :]
                    )
                    for ki in range(DKC):
                        kt = kq * DKC + ki
                        if j < 5:
                            rhs = feat_sb[:, 0, kt, :]
                        else:
                            si = j - 5
                            fa = fselp.tile([P, HB], BF16, tag="fa")
                            nc.vector.tensor_scalar(
                                fa[:],
                                feat_sb[:, 0, kt, :],
                                sel_sb[:, si, 0:1],
                                None,
                                ALU.mult,
                            )
                            fb = fselp.tile([P, HB], BF16, tag="fb")
                            nc.vector.tensor_scalar(
                                fb[:],
                                feat_sb[:, 1, kt, :],
                                sel_sb[:, si, 1:2],
                                None,
                                ALU.mult,
                            )
                            fs = fselp.tile([P, HB], BF16, tag="fs")
                            nc.vector.tensor_tensor(fs[:], fa[:], fb[:], ALU.add)
                            rhs = fs[:]
                        for db in range(DB):
                            nc.tensor.matmul(
                                pss[db][:],
                                wd_t[:, ki, db * P : (db + 1) * P],
                                rhs,
                                start=(kt == 0),
                                stop=(kt == KT - 1),
                            )
                for db in range(DB):
                    ot = outp.tile([P, HB], F32, tag="ot")
                    nc.scalar.copy(ot[:], pss[db][:])
                    nc.sync.dma_start(po_d[j, db], ot[:])

            # Issue the db-outer slot's four resident W chunk DMAs now --
            # they transfer while the fp8 slots (which don't touch the wd
            # pool or lanes) occupy the PE.
            wj_last = BF16_SLOTS.index(NSLOT - 1)
            wd_last = []
            for kq in range(DKQ):
                wd_t = wdp.tile([P, DKC, D], BF16, tag="wd", name=f"wd_last_{kq}")
                nc.sync.dma_start(
                    wd_t[:], wd_d[wj_last, :, kq * DKC : (kq + 1) * DKC, :]
                )
                wd_last.append(wd_t)

            # ------------- decode: fp8e4 DoubleRow slots -------------
            # One DR matmul contracts a kt PAIR (256 rows): lhsT [128,2,128]
            # fp8 W, rhs [128,2,512] fp8 feat (cast from bf16 on DVE).
            for si, j in enumerate(FP8_SLOTS):
                pss = [
                    psump.tile([P, HB], F32, tag="ps", name=f"ps8_{j}_{db}")
                    for db in range(DB)
                ]
                for kq in range(DKQ):
                    wd8_t = wdp8.tile([P, DKC, D], F8E4, tag="wd8")
                    nc.sync.dma_start(
                        wd8_t[:], wd8_d[si, :, kq * DKC : (kq + 1) * DKC, :]
                    )
                    for qi in range(DKC // 2):
                        kt = kq * DKC + 2 * qi
                        f8 = f8p.tile([P, 2, HB], F8E4, tag="f8")
                        nc.vector.tensor_copy(f8[:], feat_sb[:, 0, kt : kt + 2, :])
                        for db in range(DB):
                            nc.tensor.matmul(
                                pss[db][:],
                                wd8_t[:, 2 * qi : 2 * qi + 2, db * P : (db + 1) * P],
                                f8[:],
                                start=(kt == 0),
                                stop=(kt == KT - 2),
                                perf_mode=mybir.MatmulPerfMode.DoubleRow,
                            )
                for db in range(DB):
                    ot = outp.tile([P, HB], F32, tag="ot")
                    nc.scalar.copy(ot[:], pss[db][:])
                    nc.sync.dma_start(po_d[j, db], ot[:])

            # ------------- decode: last slot, db-outer -------------
            # db-outer/kt-inner so its six PSUM evictions pipeline behind
            # each db group; the final db group is further split into two
            # token halves so the kernel's tail is a [128,256] eviction +
            # 0.5MB DMA instead of a full bank.
            j = NSLOT - 1
            for db in range(DB - 1):
                ps = psump.tile([P, HB], F32, tag="ps", name=f"ps_last_{db}")
                for kt in range(KT):
                    nc.tensor.matmul(
                        ps[:],
                        wd_last[kt // DKC][:, kt % DKC, db * P : (db + 1) * P],
                        feat_sb[:, 1, kt, :],
                        start=(kt == 0),
                        stop=(kt == KT - 1),
                    )
                ot = outp.tile([P, HB], F32, tag="ot")
                nc.scalar.copy(ot[:], ps[:])
                nc.sync.dma_start(po_d[j, db], ot[:])
            db = DB - 1
            HH = HB // 2
            ps = psump.tile([P, HB], F32, tag="ps", name="ps_last_tail")
            for h in range(2):
                for kt in range(KT):
                    nc.tensor.matmul(
                        ps[:, h * HH : (h + 1) * HH],
                        wd_last[kt // DKC][:, kt % DKC, db * P : (db + 1) * P],
                        feat_sb[:, 1, kt, h * HH : (h + 1) * HH],
                        start=(kt == 0),
                        stop=(kt == KT - 1),
                    )
                ot = outp.tile([P, HH], F32, tag="ot2")
                nc.scalar.copy(ot[:], ps[:, h * HH : (h + 1) * HH])
                nc.sync.dma_start(po_d[j, db, :, h * HH : (h + 1) * HH], ot[:])

    _strip_redundant_self_waits(nc)
    # run_bass_via_pjrt serializes a prebuilt nc without finalizing it, but
    # Bacc's finalize/compile pipeline (register allocation + sync-wait
    # legalization) is required for a valid NEFF.
    nc.finalize()
    _NC_CACHE["nc"] = nc
    return nc


def _prepare_in_maps(x, W_enc, b_enc, log_threshold, W_dec, b_dec):
    x = np.ascontiguousarray(np.asarray(x, dtype=np.float32))
    W_enc = np.asarray(W_enc, dtype=np.float32)
    b_enc = np.asarray(b_enc, dtype=np.float32)
    log_threshold = np.asarray(log_threshold, dtype=np.float32)
    W_dec = np.asarray(W_dec, dtype=np.float32)

    thresh = np.exp(log_threshold)
    tmb_full = thresh                               # [8, 4096] (hb > t form)

    l_idx, k_idx = np.triu_indices(NL)
    didx = {(int(l), int(k)): i for i, (l, k) in enumerate(zip(l_idx, k_idx))}

    x_b = x.astype(NPBF16)
    W_enc_b = W_enc.astype(NPBF16)
    W_dec_b = W_dec.astype(NPBF16)

    in_maps = []
    slot_infos = []
    for c in range(NCORES):
        l, half, srcs, slots = _core_slots(c)
        tok = slice(half * HB, (half + 1) * HB)

        xt = np.empty((P, 2, DB, HB), NPBF16)
        for s, src in enumerate(srcs):
            xs = x_b[tok, src, :]                   # [HB, D]
            xt[:, s] = xs.T.reshape(DB, P, HB).transpose(1, 0, 2)

        we = np.empty((2, KC, P, KI, DB, P), NPBF16)
        for s, src in enumerate(srcs):
            w6 = W_enc_b[src].reshape(DB, P, KT, P)         # [db, p, kt, kin]
            w7 = w6.transpose(2, 1, 0, 3)                   # [kt, p, db, kin]
            we[s] = w7.reshape(KC, KI, P, DB, P).transpose(0, 2, 1, 3, 4)

        wd = np.empty((len(BF16_SLOTS), P, KT, D), NPBF16)
        for wj, j in enumerate(BF16_SLOTS):
            s, tgt = slots[j]
            w = W_dec_b[didx[(srcs[s], tgt)]]               # [K, D]
            wd[wj] = w.reshape(KT, P, D).transpose(1, 0, 2)

        wd8 = np.empty((len(FP8_SLOTS), P, KT, D), NPF8E4)
        for si, j in enumerate(FP8_SLOTS):
            s, tgt = slots[j]
            w = W_dec[didx[(srcs[s], tgt)]] * FP8_WSCALE    # [K, D] f32
            wd8[si] = w.reshape(KT, P, D).transpose(1, 0, 2).astype(NPF8E4)

        tmb = np.empty((P, 2, KT), np.float32)
        be = np.empty((P, 2, KT), np.float32)
        for s, src in enumerate(srcs):
            tmb[:, s, :] = tmb_full[src].reshape(KT, P).T
            be[:, s, :] = b_enc[src].reshape(KT, P).T

        sel = np.zeros((P, len(SEL_SLOTS), 2), np.float32)
        for si, j in enumerate(SEL_SLOTS):
            sel[:, si, slots[j][0]] = 1.0

        in_maps.append(
            {"xt": xt, "we": we, "wd": wd, "wd8": wd8, "tmb": tmb, "be": be,
             "sel": sel}
        )
        slot_infos.append((half, [(srcs[s], t) for s, t in slots]))

    return in_maps, slot_infos


def _assemble_output(results, slot_infos, b_dec):
    b_dec = np.asarray(b_dec, dtype=np.float32)
    l_idx, k_idx = np.triu_indices(NL)

    out = np.zeros((B, NL, D), np.float32)
    for c in range(NCORES):
        half, slots_abs = slot_infos[c]
        po = np.asarray(results[c]["po"], dtype=np.float32)  # [9, DB, P, HB]
        tok = slice(half * HB, (half + 1) * HB)
        for j, (_src, tgt) in enumerate(slots_abs):
            pj = po[j]
            if j in FP8_SLOTS:
                pj = pj * (1.0 / FP8_WSCALE)
            out[tok, tgt, :] += pj.reshape(D, HB).T

    bsum = np.zeros((NL, D), np.float32)
    for i in range(len(l_idx)):
        bsum[k_idx[i]] += b_dec[i]
    out += bsum[None, :, :]
    return out


def _run(x, W_enc, b_enc, log_threshold, W_dec, b_dec, trace=False, **kw):
    nc = _build_nc()
    in_maps, slot_infos = _prepare_in_maps(
        x, W_enc, b_enc, log_threshold, W_dec, b_dec
    )
    res = run_bass_kernel_spmd(nc, in_maps, list(range(NCORES)), trace=trace, **kw)
    out = _assemble_output(res.results, slot_infos, b_dec)
    return out, res


def kernel(x, W_enc, b_enc, log_threshold, W_dec, b_dec):
    out, _ = _run(x, W_enc, b_enc, log_threshold, W_dec, b_dec, trace=False)
    return out



# revision 15
# speedup vs baseline: 1.0915x; 1.0072x over previous
"""Trainium2 Bass kernel for the CLT (cross-layer transcoder) forward pass.

Problem shapes (hardcoded, from the reference):
    x:             [1024, 8, 768]   f32
    W_enc:         [8, 768, 4096]   f32
    b_enc:         [8, 4096]        f32
    log_threshold: [8, 4096]        f32
    W_dec:         [36, 4096, 768]  f32   (36 = triu pairs of 8 layers)
    b_dec:         [36, 768]        f32
    out:           [1024, 8, 768]   f32

Math:
    hidden[b,n,k] = x[b,n,:] @ W_enc[n] + b_enc[n]
    feat = hidden * (hidden > exp(log_threshold))        (JumpReLU)
    out[:,k,:] = sum_{l<=k} feat[:,l,:] @ W_dec[pair(l,k)] + b_dec sums

Sharding (8 cores, single uniform SPMD program):
    Work units (1 unit = a [1024,768]x[768,4096]-sized matmul): encode 8,
    decode 36, total 44 -> 5.5 per core.  Core c handles sources
    (l, 7-l) with l = c//2, for token half c%2 (512 tokens).  That gives
    every core exactly 2 half-encodes (1 unit) + 9 half-pair decodes
    (4.5 units) -- a perfectly balanced, duplication-free split.

    Decode slot j of a core reads the feat of its source 0 (j < 8-l) or
    source 1 (else).  Slots 0-4 are always source 0 and slot 8 always
    source 1; slots 5-7 vary per core, so their matmul rhs is built as
    feat0*c0 + feat1*c1 with per-core 0/1 coefficients shipped as data,
    keeping the compiled program identical on all 8 cores.

    All matmuls run in bf16 (inputs cast on host) with fp32 PSUM
    accumulation.  Per-slot partial outputs [768, 512] go back to the
    host, which transposes/sums them into the full [1024, 8, 768] output
    (plus the b_dec per-target bias sums).

    fp8 DoubleRow: decode slots 3 and 4 (8 of the 36 pairs, uniform
    across cores) run as fp8e4 DoubleRow matmuls -- W_dec pre-scaled by
    64 into e4m3 on the host (values would hit subnormals otherwise),
    feat cast bf16->e4m3 on DVE per kt-pair.  PE throughput for those
    slots is ~1.44x bf16; measured global rel err 1.82e-2 < 2e-2 gate.
    Slot order: bf16 kt-outer slots first, then the two fp8 slots, then
    the db-outer bf16 slot 8 -- while the fp8 slots run, the wd pool is
    idle so slot 8's four resident W chunks prefetch without stalling.
"""

import os
import sys

for _p in ("/opt/trn_rl_repo", "/root/.axon_site/_ro/trn_rl_repo"):
    if os.path.isdir(_p) and _p not in sys.path:
        sys.path.insert(0, _p)

import ml_dtypes
import numpy as np

import concourse.bass as bass
import concourse.mybir as mybir
import concourse.tile as tile
from concourse import bacc
from concourse.bass_utils import run_bass_kernel_spmd

BF16 = mybir.dt.bfloat16
F32 = mybir.dt.float32
F8E4 = mybir.dt.float8e4
NPBF16 = ml_dtypes.bfloat16
NPF8E4 = ml_dtypes.float8_e4m3

B, NL, D, K = 1024, 8, 768, 4096
HB = B // 2          # tokens per half (per core)
P = 128
DB = D // P          # 6 d-tiles
KT = K // P          # 32 k-tiles
KI = 4               # k-tiles per W_enc DMA chunk
KC = KT // KI        # 8 W_enc chunks
DKC = 8              # k-tiles per W_dec DMA chunk
DKQ = KT // DKC      # 4 W_dec chunks per decoder
NSLOT = 9            # decode half-pairs per core
SEL_SLOTS = (5, 6, 7)  # slots whose source varies per core
FP8_SLOTS = (3, 4)   # slots decoded via fp8e4 DoubleRow (src 0 for all cores)
FP8_WSCALE = 64.0    # host pre-scale on fp8 W_dec (undone on the host)
BF16_SLOTS = (0, 1, 2, 5, 6, 7, 8)   # wd dram-tensor index = position here
KT_ORDER = (0, 1, 2, 5, 6, 7)        # bf16 kt-outer slots, processed first
NCORES = 8

AF = mybir.ActivationFunctionType
ALU = mybir.AluOpType

_NC_CACHE = {}


def _install_dma_lane_pinning():
    """Pin each DMA stream to a fixed DMAHW lane.

    Tile round-robins HWDGE DMAs over 8 DMAHW semaphore lanes.  A DMA that
    reuses an SBUF slot then needs waits on (a) the PE readers of the slot
    (WAR), (b) the previous writer's lane sem (WAW), and (c) its own lane's
    predecessor (in-order completion per sem) -- three sync waits, but the
    walrus DMA instruction struct only encodes two.  Pinning a whole stream
    (all W_enc chunks, all W_dec chunks, ...) to one lane merges (b) and
    (c) into a single semaphore wait, guaranteeing <=2 waits per DMA.
    """
    import concourse.tile_sem_assignment as tsa

    if getattr(tsa, "_clt_lane_pinned", False):
        return

    _orig = tsa.TileClockTick._assign_tick

    def _dma_names(inst):
        names = set()
        for a in list(inst.ins) + list(inst.outs):
            t = None
            for chain in ("bass_ap", None):
                try:
                    obj = getattr(a, chain) if chain else a
                    t = obj.tensor
                    break
                except AttributeError:
                    continue
            if t is not None:
                try:
                    names.add(t.name)
                except AttributeError:
                    pass
        return names

    # Two lanes per stream: consecutive same-stream DMAs alternate lanes so
    # their issues don't head-block the Sync queue on each other's
    # completion, while slot-reuse (WAW) partners still land on the SAME
    # lane because the lane count (2) divides the pool bufs (4).
    # xt spreads over the po lanes too (idle at kernel start); wd8 rides
    # the const lanes (idle once the decode phase reaches the fp8 slots).
    _LANES = {
        "we": [0, 4],
        "wd": [1, 5],
        "po": [2, 6],
        "wd8": [3, 7],
        "xt": [3, 7, 2, 6],
        "_const": [3, 7],
    }

    def _assign_tick(self, inst):
        if isinstance(inst, tsa.DMAInst) and inst.engine != mybir.EngineType.Pool:
            names = _dma_names(inst)
            stream = "_const"
            for key in ("we", "wd8", "wd", "po", "xt"):
                if key in names:
                    stream = key
                    break
            ctr = getattr(self, "_clt_lane_ctr", None)
            if ctr is None:
                ctr = {}
                self._clt_lane_ctr = ctr
            lanes = _LANES[stream]
            i = ctr.get(stream, 0)
            ctr[stream] = i + 1
            self.next_hw_dma_idx = lanes[i % len(lanes)]
        return _orig(self, inst)

    tsa.TileClockTick._assign_tick = _assign_tick
    tsa._clt_lane_pinned = True


def _core_slots(c):
    """Return (l, half, srcs, slots) for core c.

    slots: list of (local_source_index, target_layer); first 8-l entries
    use local source 0 (= layer l), the rest local source 1 (= layer 7-l).
    """
    l, half = c // 2, c % 2
    srcs = (l, 7 - l)
    slots = [(0, t) for t in range(l, 8)] + [(1, t) for t in range(7 - l, 8)]
    assert len(slots) == NSLOT
    for j, (s, _) in enumerate(slots):
        if j < 5:
            assert s == 0
        elif j == 8:
            assert s == 1
    return l, half, srcs, slots


def _strip_redundant_self_waits(nc):
    """Drop trivially-satisfied same-engine semaphore waits.

    Tile sometimes emits a wait on an engine's own semaphore for a value
    the engine has necessarily already passed (its in-order predecessors
    increment that sem on completion).  Such waits are runtime no-ops but
    consume one of the 1-2 sync-wait slots a walrus instruction struct can
    encode, overflowing the encoder.  Keep a pipeline-depth margin: a wait
    is dropped only if satisfied even with queue_depth instructions still
    in flight at sequencer dispatch time.
    """
    import re
    from collections import defaultdict

    # Engine completion sems are named like PE_44 / DVE_44 / Activation_44.
    # Only those are safe to treat as "own engine program order" - barrier
    # and event sems must never be touched.
    _ENG_SEM_RE = {
        mybir.EngineType.PE: re.compile(r"^PE_\d+$"),
        mybir.EngineType.DVE: re.compile(r"^DVE_\d+$"),
        mybir.EngineType.Activation: re.compile(r"^Activation_\d+$"),
    }
    _STRIP_TYPES = (
        "InstTensorScalarPtr",
        "InstTensorScalar",
        "InstTensorTensor",
        "InstTensorCopy",
        "InstActivation",
        "InstMatmult",
        "InstLdweights",
    )

    margins = defaultdict(lambda: 12)
    margins[mybir.EngineType.PE] = 80

    cum = defaultdict(int)
    dropped = 0
    for bb in nc.m.functions[0].blocks:
        for ins in bb.instructions:
            si = ins.sync_info
            if si is None:
                continue
            sem_re = _ENG_SEM_RE.get(ins.engine)
            if type(ins).__name__ in _STRIP_TYPES and sem_re is not None:
                margin = margins[ins.engine]
                kept = []
                for w in si.on_wait:
                    if (
                        sem_re.match(w.ant_name)
                        and w.wait_mode == "sem-ge-imm"
                        and w.wait_value <= cum[w.ant_name] - margin
                    ):
                        dropped += 1
                        continue
                    kept.append(w)
                if len(kept) != len(si.on_wait):
                    ins.sync_info = mybir.SyncInfo(
                        on_wait=kept, on_update=si.on_update
                    )
                    si = ins.sync_info
            for u in si.on_update:
                cum[u.ant_name] += u.update_value
    return dropped


def _build_nc():
    if "nc" in _NC_CACHE:
        return _NC_CACHE["nc"]

    # Bacc (not raw Bass): its compile pipeline legalizes sync waits down
    # to the 1-wait-per-instruction TRN2 limit via event semaphores.
    _install_dma_lane_pinning()
    nc = bacc.Bacc()

    xt_d = nc.dram_tensor("xt", [P, 2, DB, HB], BF16, kind="ExternalInput")
    we_d = nc.dram_tensor("we", [2, KC, P, KI, DB, P], BF16, kind="ExternalInput")
    wd_d = nc.dram_tensor(
        "wd", [len(BF16_SLOTS), P, KT, D], BF16, kind="ExternalInput"
    )
    wd8_d = nc.dram_tensor(
        "wd8", [len(FP8_SLOTS), P, KT, D], F8E4, kind="ExternalInput"
    )
    tmb_d = nc.dram_tensor("tmb", [P, 2, KT], F32, kind="ExternalInput")
    be_d = nc.dram_tensor("be", [P, 2, KT], F32, kind="ExternalInput")
    sel_d = nc.dram_tensor("sel", [P, len(SEL_SLOTS), 2], F32, kind="ExternalInput")
    po_d = nc.dram_tensor("po", [NSLOT, DB, P, HB], F32, kind="ExternalOutput")

    with tile.TileContext(nc) as tc:
        with (
            tc.tile_pool(name="const", bufs=1) as constp,
            tc.tile_pool(name="wep", bufs=4) as wep,
            tc.tile_pool(name="wdp", bufs=4) as wdp,
            tc.tile_pool(name="wdp8", bufs=3) as wdp8,
            tc.tile_pool(name="featp", bufs=1) as featp,
            tc.tile_pool(name="fselp", bufs=4) as fselp,
            tc.tile_pool(name="f8p", bufs=6) as f8p,
            tc.tile_pool(name="tmpp", bufs=4) as tmpp,
            tc.tile_pool(name="outp", bufs=4) as outp,
            tc.tile_pool(name="psum", bufs=8, space="PSUM") as psump,
        ):
            # Head: a logical DMA spreads across all 16 HW queues, so big
            # transfers are fast -- the serial per-issue cost (~0.6us) on
            # the Sync engine dominates instead.  Issue xt source-0 as ONE
            # DMA, and split only the first W_enc chunk per-ki so the
            # first matmul group waits on a 196KB piece, not 786KB.
            xt_sb = constp.tile([P, 2, DB, HB], BF16)
            nc.sync.dma_start(xt_sb[:, 0], xt_d[:, 0])
            we_t0 = wep.tile([P, KI, DB, P], BF16, tag="we")
            for ki in range(KI):
                nc.sync.dma_start(we_t0[:, ki], we_d[0, 0, :, ki])
            we_t1 = wep.tile([P, KI, DB, P], BF16, tag="we")
            nc.sync.dma_start(we_t1[:], we_d[0, 1])
            nc.sync.dma_start(xt_sb[:, 1], xt_d[:, 1])
            tmb_sb = constp.tile([P, 2, KT], F32)
            nc.sync.dma_start(tmb_sb[:], tmb_d[:])
            be_sb = constp.tile([P, 2, KT], F32)
            nc.sync.dma_start(be_sb[:], be_d[:])
            sel_sb = constp.tile([P, len(SEL_SLOTS), 2], F32)
            nc.sync.dma_start(sel_sb[:], sel_d[:])

            # One-time "absorb" ops: the first DVE/ACT instructions that use
            # an AP-scalar operand (TensorScalarPtr / ActivationPtr) can
            # encode only ONE sync wait, but they'd otherwise have to wait on
            # both the PSUM producer (PE sem) and the const-DMA (DMAHW sem).
            # Touch each DMA-loaded const from both engines up front so the
            # engines' vector clocks already cover the DMAs.
            probe = constp.tile([P, 4], F32)
            nc.vector.tensor_copy(probe[:, 0:1], tmb_sb[:, 0, 0:1])
            nc.vector.tensor_copy(probe[:, 1:2], sel_sb[:, 0, 0:1])
            nc.scalar.copy(probe[:, 2:3], be_sb[:, 0, 0:1])

            feat_sb = featp.tile([P, 2, KT, HB], BF16)

            # ---------------- encode ----------------
            for s in range(2):
                for kc in range(KC):
                    if s == 0 and kc == 0:
                        we_t = we_t0
                    elif s == 0 and kc == 1:
                        we_t = we_t1
                    else:
                        we_t = wep.tile([P, KI, DB, P], BF16, tag="we")
                        nc.sync.dma_start(we_t[:], we_d[s, kc])
                    for ki in range(KI):
                        kt = kc * KI + ki
                        ps = psump.tile([P, HB], F32, tag="ps")
                        for db in range(DB):
                            nc.tensor.matmul(
                                ps[:],
                                we_t[:, ki, db, :],
                                xt_sb[:, s, db, :],
                                start=(db == 0),
                                stop=(db == DB - 1),
                            )
                        # JumpReLU: hb = h + b on ACT (sole PSUM reader, so
                        # the next matmul group's WAR is a single wait);
                        # mask + mult on DVE read the bf16 hb at 2x rate.
                        hb_t = tmpp.tile([P, HB], BF16, tag="hb")
                        nc.scalar.activation(
                            hb_t[:], ps[:], AF.Identity, bias=be_sb[:, s, kt : kt + 1]
                        )
                        mask = tmpp.tile([P, HB], BF16, tag="mask")
                        nc.vector.tensor_scalar(
                            mask[:], hb_t[:], tmb_sb[:, s, kt : kt + 1], None, ALU.is_gt
                        )
                        nc.vector.tensor_tensor(
                            feat_sb[:, s, kt, :], hb_t[:], mask[:], ALU.mult
                        )

            # ---------------- decode: bf16 kt-outer slots ----------------
            for j in KT_ORDER:
                wj = BF16_SLOTS.index(j)
                pss = [
                    psump.tile([P, HB], F32, tag="ps", name=f"ps_{j}_{db}")
                    for db in range(DB)
                ]
                for kq in range(DKQ):
                    wd_t = wdp.tile([P, DKC, D], BF16, tag="wd")
                    nc.sync.dma_start(
                        wd_t[:], wd_d[wj, :, kq * DKC : (kq + 1) * DKC, :]
                    )
                    for ki in range(DKC):
                        kt = kq * DKC + ki
                        if j < 5:
                            rhs = feat_sb[:, 0, kt, :]
                        else:
                            si = j - 5
                            fa = fselp.tile([P, HB], BF16, tag="fa")
                            nc.vector.tensor_scalar(
                                fa[:],
                                feat_sb[:, 0, kt, :],
                                sel_sb[:, si, 0:1],
                                None,
                                ALU.mult,
                            )
                            fb = fselp.tile([P, HB], BF16, tag="fb")
                            nc.vector.tensor_scalar(
                                fb[:],
                                feat_sb[:, 1, kt, :],
                                sel_sb[:, si, 1:2],
                                None,
                                ALU.mult,
                            )
                            fs = fselp.tile([P, HB], BF16, tag="fs")
                            nc.vector.tensor_tensor(fs[:], fa[:], fb[:], ALU.add)
                            rhs = fs[:]
                        for db in range(DB):
                            nc.tensor.matmul(
                                pss[db][:],
                                wd_t[:, ki, db * P : (db + 1) * P],
                                rhs,
                                start=(kt == 0),
                                stop=(kt == KT - 1),
                            )
                for db in range(DB):
                    ot = outp.tile([P, HB], F32, tag="ot")
                    nc.scalar.copy(ot[:], pss[db][:])
                    nc.sync.dma_start(po_d[j, db], ot[:])

            # Issue the db-outer slot's four resident W chunk DMAs now --
            # they transfer while the fp8 slots (which don't touch the wd
            # pool or lanes) occupy the PE.
            wj_last = BF16_SLOTS.index(NSLOT - 1)
            wd_last = []
            for kq in range(DKQ):
                wd_t = wdp.tile([P, DKC, D], BF16, tag="wd", name=f"wd_last_{kq}")
                nc.sync.dma_start(
                    wd_t[:], wd_d[wj_last, :, kq * DKC : (kq + 1) * DKC, :]
                )
                wd_last.append(wd_t)

            # ------------- decode: fp8e4 DoubleRow slots -------------
            # One DR matmul contracts a kt PAIR (256 rows): lhsT [128,2,128]
            # fp8 W, rhs [128,2,512] fp8 feat (cast from bf16 on DVE).
            for si, j in enumerate(FP8_SLOTS):
                pss = [
                    psump.tile([P, HB], F32, tag="ps", name=f"ps8_{j}_{db}")
                    for db in range(DB)
                ]
                for kq in range(DKQ):
                    wd8_t = wdp8.tile([P, DKC, D], F8E4, tag="wd8")
                    nc.sync.dma_start(
                        wd8_t[:], wd8_d[si, :, kq * DKC : (kq + 1) * DKC, :]
                    )
                    for qi in range(DKC // 2):
                        kt = kq * DKC + 2 * qi
                        f8 = f8p.tile([P, 2, HB], F8E4, tag="f8")
                        nc.vector.tensor_copy(f8[:], feat_sb[:, 0, kt : kt + 2, :])
                        for db in range(DB):
                            nc.tensor.matmul(
                                pss[db][:],
                                wd8_t[:, 2 * qi : 2 * qi + 2, db * P : (db + 1) * P],
                                f8[:],
                                start=(kt == 0),
                                stop=(kt == KT - 2),
                                perf_mode=mybir.MatmulPerfMode.DoubleRow,
                            )
                for db in range(DB):
                    ot = outp.tile([P, HB], F32, tag="ot")
                    nc.scalar.copy(ot[:], pss[db][:])
                    nc.sync.dma_start(po_d[j, db], ot[:])

            # ------------- decode: last slot, db-outer -------------
            # db-outer/kt-inner so its six PSUM evictions pipeline behind
            # each db group; the final db group is further split into two
            # token halves so the kernel's tail is a [128,256] eviction +
            # 0.5MB DMA instead of a full bank.
            j = NSLOT - 1
            for db in range(DB - 1):
                ps = psump.tile([P, HB], F32, tag="ps", name=f"ps_last_{db}")
                for kt in range(KT):
                    nc.tensor.matmul(
                        ps[:],
                        wd_last[kt // DKC][:, kt % DKC, db * P : (db + 1) * P],
                        feat_sb[:, 1, kt, :],
                        start=(kt == 0),
                        stop=(kt == KT - 1),
                    )
                ot = outp.tile([P, HB], F32, tag="ot")
                nc.scalar.copy(ot[:], ps[:])
                nc.sync.dma_start(po_d[j, db], ot[:])
            db = DB - 1
            HH = HB // 2
            ps = psump.tile([P, HB], F32, tag="ps", name="ps_last_tail")
            for h in range(2):
                for kt in range(KT):
                    nc.tensor.matmul(
                        ps[:, h * HH : (h + 1) * HH],
                        wd_last[kt // DKC][:, kt % DKC, db * P : (db + 1) * P],
                        feat_sb[:, 1, kt, h * HH : (h + 1) * HH],
                        start=(kt == 0),
                        stop=(kt == KT - 1),
                    )
                ot = outp.tile([P, HH], F32, tag="ot2")
                nc.scalar.copy(ot[:], ps[:, h * HH : (h + 1) * HH])
                nc.sync.dma_start(po_d[j, db, :, h * HH : (h + 1) * HH], ot[:])

    _strip_redundant_self_waits(nc)
    # run_bass_via_pjrt serializes a prebuilt nc without finalizing it, but
    # Bacc's finalize/compile pipeline (register allocation + sync-wait
    # legalization) is required for a valid NEFF.
    nc.finalize()
    _NC_CACHE["nc"] = nc
    return nc


def _prepare_in_maps(x, W_enc, b_enc, log_threshold, W_dec, b_dec):
    x = np.ascontiguousarray(np.asarray(x, dtype=np.float32))
    W_enc = np.asarray(W_enc, dtype=np.float32)
    b_enc = np.asarray(b_enc, dtype=np.float32)
    log_threshold = np.asarray(log_threshold, dtype=np.float32)
    W_dec = np.asarray(W_dec, dtype=np.float32)

    thresh = np.exp(log_threshold)
    tmb_full = thresh                               # [8, 4096] (hb > t form)

    l_idx, k_idx = np.triu_indices(NL)
    didx = {(int(l), int(k)): i for i, (l, k) in enumerate(zip(l_idx, k_idx))}

    x_b = x.astype(NPBF16)
    W_enc_b = W_enc.astype(NPBF16)
    W_dec_b = W_dec.astype(NPBF16)

    in_maps = []
    slot_infos = []
    for c in range(NCORES):
        l, half, srcs, slots = _core_slots(c)
        tok = slice(half * HB, (half + 1) * HB)

        xt = np.empty((P, 2, DB, HB), NPBF16)
        for s, src in enumerate(srcs):
            xs = x_b[tok, src, :]                   # [HB, D]
            xt[:, s] = xs.T.reshape(DB, P, HB).transpose(1, 0, 2)

        we = np.empty((2, KC, P, KI, DB, P), NPBF16)
        for s, src in enumerate(srcs):
            w6 = W_enc_b[src].reshape(DB, P, KT, P)         # [db, p, kt, kin]
            w7 = w6.transpose(2, 1, 0, 3)                   # [kt, p, db, kin]
            we[s] = w7.reshape(KC, KI, P, DB, P).transpose(0, 2, 1, 3, 4)

        wd = np.empty((len(BF16_SLOTS), P, KT, D), NPBF16)
        for wj, j in enumerate(BF16_SLOTS):
            s, tgt = slots[j]
            w = W_dec_b[didx[(srcs[s], tgt)]]               # [K, D]
            wd[wj] = w.reshape(KT, P, D).transpose(1, 0, 2)

        wd8 = np.empty((len(FP8_SLOTS), P, KT, D), NPF8E4)
        for si, j in enumerate(FP8_SLOTS):
            s, tgt = slots[j]
            w = W_dec[didx[(srcs[s], tgt)]] * FP8_WSCALE    # [K, D] f32
            wd8[si] = w.reshape(KT, P, D).transpose(1, 0, 2).astype(NPF8E4)

        tmb = np.empty((P, 2, KT), np.float32)
        be = np.empty((P, 2, KT), np.float32)
        for s, src in enumerate(srcs):
            tmb[:, s, :] = tmb_full[src].reshape(KT, P).T
            be[:, s, :] = b_enc[src].reshape(KT, P).T

        sel = np.zeros((P, len(SEL_SLOTS), 2), np.float32)
        for si, j in enumerate(SEL_SLOTS):
            sel[:, si, slots[j][0]] = 1.0

        in_maps.append(
            {"xt": xt, "we": we, "wd": wd, "wd8": wd8, "tmb": tmb, "be": be,
             "sel": sel}
        )
        slot_infos.append((half, [(srcs[s], t) for s, t in slots]))

    return in_maps, slot_infos


def _assemble_output(results, slot_infos, b_dec):
    b_dec = np.asarray(b_dec, dtype=np.float32)
    l_idx, k_idx = np.triu_indices(NL)

    out = np.zeros((B, NL, D), np.float32)
    for c in range(NCORES):
        half, slots_abs = slot_infos[c]
        po = np.asarray(results[c]["po"], dtype=np.float32)  # [9, DB, P, HB]
        tok = slice(half * HB, (half + 1) * HB)
        for j, (_src, tgt) in enumerate(slots_abs):
            pj = po[j]
            if j in FP8_SLOTS:
                pj = pj * (1.0 / FP8_WSCALE)
            out[tok, tgt, :] += pj.reshape(D, HB).T

    bsum = np.zeros((NL, D), np.float32)
    for i in range(len(l_idx)):
        bsum[k_idx[i]] += b_dec[i]
    out += bsum[None, :, :]
    return out


def _run(x, W_enc, b_enc, log_threshold, W_dec, b_dec, trace=False, **kw):
    nc = _build_nc()
    in_maps, slot_infos = _prepare_in_maps(
        x, W_enc, b_enc, log_threshold, W_dec, b_dec
    )
    res = run_bass_kernel_spmd(nc, in_maps, list(range(NCORES)), trace=trace, **kw)
    out = _assemble_output(res.results, slot_infos, b_dec)
    return out, res


def kernel(x, W_enc, b_enc, log_threshold, W_dec, b_dec):
    out, _ = _run(x, W_enc, b_enc, log_threshold, W_dec, b_dec, trace=False)
    return out

